# revision 1
# baseline (speedup 1.0000x reference)
"""Trainium2 Bass kernel for nn_BinaryDilGroupConv.

Reference computation (B=32, C=256, H=W=56, GROUPS=4):
    c1  = conv2d(sign(x), sign(w1), stride=2, pad=1, groups=4)   # -> (B,256,28,28)
    x1  = batchnorm_train(c1, g1, b1) + maxpool3x3s2p1(x)
    c2  = conv2d(sign(x1), sign(w2), 1x1)
    out = batchnorm_train(c2, g2, b2) + x1

Strategy: data-parallel over batch across 8 NeuronCores (4 images/core).
BatchNorm batch statistics are all-reduced across cores (one tiny
[128,4] f32 AllReduce per stage); a dummy warmup AllReduce issued at
kernel start absorbs the collective path's cold-start / multi-core
launch-skew latency concurrently with the input DMA + conv1 phase.
Weights are sign-binarized, transposed and block-diag packed on the host
(they are tiny).  conv1 consumes a true sign(x) in bf16 (exact +-1/0);
conv2 consumes a true sign(x1) produced by the Scalar engine (exact
because all conv accumulations are small integers held in fp32 PSUM).
Phase C is balanced across engines: ACT does the BN1 affine and the
sign, DVE adds the maxpool shortcut and computes conv2 stats, PE runs
conv2; the second output-half's conv2 results stay PSUM-resident and
are consumed directly by the output affine.
"""

import sys

for _p in ("/opt/trn_rl_repo", "/root/.axon_site/_ro/trn_rl_repo"):
    if _p not in sys.path:
        sys.path.append(_p)

import numpy as np
import ml_dtypes

import concourse.bass as bass
import concourse.bacc as bacc
import concourse.mybir as mybir
import concourse.tile as tile
from concourse import bass_utils

N_CORES = 8
B, C, H, W = 32, 256, 56, 56
BL = B // N_CORES          # images per core
OH = OW = 28
NPIX = OH * OW             # 784
NLOC = BL * NPIX           # samples/channel for local stats (3136)
NGLB = B * NPIX            # samples/channel for global stats (25088)
EPS = 1e-5

F32 = mybir.dt.float32
F16 = mybir.dt.float16
BF16 = mybir.dt.bfloat16

# conv1 kernel-position order: center first (start=True covers the full
# output rect), (2,2) last (stop=True covers the full rect too).
KPOS_ORDER = [(1, 1), (0, 0), (0, 1), (0, 2), (1, 0), (1, 2), (2, 0), (2, 1), (2, 2)]

# vecs columns
VG1, VB1 = 0, 1            # +2*h
VG2, VB2 = 4, 5            # +2*co
VK2, VK2B = 8, 10          # +co

RG = [list(range(N_CORES))]


def _emit_conv1(nc, ps, sx, w1t, h):
    for kh, kw in KPOS_ORDER:
        i0 = 1 if kh == 0 else 0
        j0 = 1 if kw == 0 else 0
        ncol = 28 - j0
        woff = ((kh * 3 + kw) * 2 + h) * 128
        for b in range(2):
            r0 = max(i0, 14 * b)
            nr = 14 * b + 14 - r0
            a0 = 2 * r0 + kh - 1
            c0 = 2 * j0 + kw - 1
            bank = ps[:, 512 * b:512 * b + 392].rearrange("p (r c) -> p r c", c=28)
            out_v = bank[:, r0 - 14 * b:r0 - 14 * b + nr, j0:28]
            rhs = sx[:, a0:a0 + 2 * (nr - 1) + 1:2, c0:c0 + 2 * (ncol - 1) + 1:2]
            nc.tensor.matmul(
                out_v,
                w1t[:, woff:woff + 128],
                rhs,
                start=(kh == 1 and kw == 1),
                stop=(kh == 2 and kw == 2),
            )


def _affine_from_sq(nc, vp, s_ap, q_ap, g_ap, b_ap, s_out, b_out, tag,
                    w=1):
    """s_out/b_out [128,w]: BN affine from global sum/sumsq."""
    mg = vp.tile([128, w], F32, name="mg", tag=f"mg{tag}")
    nc.vector.tensor_scalar_mul(mg[:], s_ap, 1.0 / NGLB)
    e2 = vp.tile([128, w], F32, name="e2", tag=f"e2{tag}")
    nc.vector.tensor_scalar_mul(e2[:], q_ap, 1.0 / NGLB)
    m2 = vp.tile([128, w], F32, name="m2", tag=f"m2{tag}")
    nc.vector.tensor_mul(m2[:], mg[:], mg[:])
    vr = vp.tile([128, w], F32, name="vr", tag=f"vr{tag}")
    nc.vector.tensor_sub(vr[:], e2[:], m2[:])
    nc.vector.tensor_scalar_add(vr[:], vr[:], EPS)
    sd = vp.tile([128, w], F32, name="sd", tag=f"sd{tag}")
    nc.scalar.sqrt(sd[:], vr[:])
    inv = vp.tile([128, w], F32, name="inv", tag=f"inv{tag}")
    nc.vector.reciprocal(inv[:], sd[:])
    nc.vector.tensor_mul(s_out, inv[:], g_ap)
    t2 = vp.tile([128, w], F32, name="t2", tag=f"t2{tag}")
    nc.vector.tensor_mul(t2[:], mg[:], s_out)
    nc.vector.tensor_sub(b_out, b_ap, t2[:])


def _build():
    nc = bacc.Bacc(
        "TRN2",
        target_bir_lowering=False,
        debug=False,
        enable_asserts=False,
        num_devices=N_CORES,
    )
    xs = nc.dram_tensor("xs", [BL, C, H, W], F32, kind="ExternalInput")
    w1b = nc.dram_tensor("w1b", [128, 2304], BF16, kind="ExternalInput")
    w2b = nc.dram_tensor("w2b", [128, 512], BF16, kind="ExternalInput")
    vecs_d = nc.dram_tensor("vecs", [128, 12], F32, kind="ExternalInput")
    out_d = nc.dram_tensor("out", [BL, C, OH, OW], F32, kind="ExternalOutput")

    xs_ap = xs.ap()
    out_ap = out_d.ap()

    with tile.TileContext(nc) as tc:
        with tc.tile_pool(name="wp", bufs=1) as wp, \
             tc.tile_pool(name="xp", bufs=3) as xp, \
             tc.tile_pool(name="sxp", bufs=3) as sxp, \
             tc.tile_pool(name="tp", bufs=2) as tp, \
             tc.tile_pool(name="mp", bufs=8) as mp, \
             tc.tile_pool(name="c1p", bufs=8) as c1p, \
             tc.tile_pool(name="x1p", bufs=8) as x1p, \
             tc.tile_pool(name="sx1p", bufs=8) as sx1p, \
             tc.tile_pool(name="c2p", bufs=8) as c2p, \
             tc.tile_pool(name="outp", bufs=6) as outp, \
             tc.tile_pool(name="vp", bufs=2) as vp, \
             tc.tile_pool(name="pp", bufs=4, space="PSUM") as pp, \
             tc.tile_pool(name="dramp", bufs=1, space="DRAM") as dramp:

            # ---- warmup collective: absorbs the ncfw cold-start latency
            # while the x DMAs stream in.  All small/constant DMAs ride the
            # Scalar HWDGE ring so the Sync ring belongs to the x loads.
            with tc.high_priority():
                wmt = wp.tile([128, 1], F32)
                nc.gpsimd.memset(wmt[:], 0.0)
                warm_in = dramp.tile([128, 1], F32)
                warm_out = dramp.tile([128, 1], F32)
                nc.gpsimd.dma_start(warm_in[:], wmt[:])
                nc.gpsimd.collective_compute(
                    "AllReduce", mybir.AluOpType.add, replica_groups=RG,
                    ins=[warm_in.opt()], outs=[warm_out.opt()])

            # ---- constants ----
            w1t = wp.tile([128, 2304], BF16)
            nc.scalar.dma_start(w1t[:], w1b.ap())
            w2t = wp.tile([128, 512], BF16)
            nc.scalar.dma_start(w2t[:], w2b.ap())
            vecs = wp.tile([128, 12], F32)
            nc.scalar.dma_start(vecs[:], vecs_d.ap())

            # stats: [p, half/co, chunk(2n+b), 6]
            st1 = wp.tile([128, 2, 8, 6], F32)
            st2 = wp.tile([128, 2, 8, 6], F32)
            s1v = wp.tile([128, 2], F32)
            b1v = wp.tile([128, 2], F32)
            s2v = wp.tile([128, 2], F32)
            b2e = wp.tile([128, 2], F32)
            agg1 = wp.tile([128, 2, 2], F32)
            agg2 = wp.tile([128, 2, 2], F32)
            pk1 = wp.tile([128, 2, 2], F32)
            pk2 = wp.tile([128, 2, 2], F32)
            g1pk = wp.tile([128, 2, 2], F32)
            g2pk = wp.tile([128, 2, 2], F32)

            m_t = {}
            c1_t = {}
            x1_t = {}
            sx1_t = {}
            c2_t = {}

            # ======== phase A (per half h): load, sign, maxpool, conv1,
            # stats -> per-half AllReduce overlapped with the other half ====
            def stage_a(h, n):
                xt = xp.tile([128, H, W], F32, name="xt")
                nc.sync.dma_start(xt[:], xs_ap[n, 128 * h:128 * h + 128])
                sx = sxp.tile([128, H, W], BF16, name="sx")
                nc.scalar.sign(sx[:], xt[:])

                # maxpool 3x3 s2 p1 (separable, boundary-safe)
                t = tp.tile([128, OH, W], F32, name="t")
                nc.vector.tensor_max(t[:], xt[:, 0:56:2, :], xt[:, 1:56:2, :])
                nc.vector.tensor_max(t[:, 1:28, :], t[:, 1:28, :], xt[:, 1:54:2, :])
                m = mp.tile([128, OH, OW], F32, name="m")
                nc.vector.tensor_max(m[:], t[:, :, 0:56:2], t[:, :, 1:56:2])
                nc.vector.tensor_max(m[:, :, 1:28], m[:, :, 1:28], t[:, :, 1:54:2])
                m_t[(n, h)] = m

                ps = pp.tile([128, 1024], F32, name="ps", tag="ps")
                _emit_conv1(nc, ps, sx, w1t, h)
                for b in range(2):
                    nc.vector.bn_stats(
                        st1[:, h, 2 * n + b], ps[:, 512 * b:512 * b + 392])
                c1 = c1p.tile([128, NPIX], F16, name="c1")
                nc.scalar.copy(
                    c1.rearrange("p (b x) -> p b x", b=2),
                    ps.rearrange("p (b x) -> p b x", b=2)[:, :, 0:392])
                c1_t[(n, h)] = c1

            def ar1():
              with tc.high_priority():
                for h in range(2):
                    nc.vector.bn_aggr(agg1[:, h], st1[:, h])
                    nc.vector.tensor_scalar_mul(
                        pk1[:, h, 0:1], agg1[:, h, 0:1], float(NLOC))
                    tq = vp.tile([128, 1], F32, name="tq", tag=f"tq1{h}")
                    nc.vector.tensor_mul(tq[:], agg1[:, h, 0:1], agg1[:, h, 0:1])
                    nc.vector.tensor_add(tq[:], tq[:], agg1[:, h, 1:2])
                    nc.vector.tensor_scalar_mul(pk1[:, h, 1:2], tq[:], float(NLOC))
                ain = dramp.tile([128, 4], F32, name="ar1in")
                aout = dramp.tile([128, 4], F32, name="ar1out")
                nc.sync.dma_start(ain[:], pk1.rearrange("p a b -> p (a b)"))
                nc.gpsimd.collective_compute(
                    "AllReduce", mybir.AluOpType.add, replica_groups=RG,
                    ins=[ain.opt()], outs=[aout.opt()])
                nc.sync.dma_start(g1pk.rearrange("p a b -> p (a b)"), aout[:])
                _affine_from_sq(
                    nc, vp, g1pk[:, :, 0], g1pk[:, :, 1],
                    vecs[:, VG1:VG1 + 3:2], vecs[:, VB1:VB1 + 3:2],
                    s1v[:], b1v[:], tag="a1", w=2)

            for h in range(2):
                for n in range(BL):
                    stage_a(h, n)
            ar1()

            # ======== phase C: x1' = s1*c1 + m, sx1 = {0,2}(x1'>=-b1'),
            # conv2 per output half, per-half stats AllReduce ==============
            def stage_c1(h, n):
                # x1 = s1*c1 + b1 + m: ACT does the affine, DVE adds the
                # shortcut, ACT produces the exact bf16 sign for conv2.
                x1 = x1p.tile([128, NPIX], F32, name="x1")
                nc.scalar.activation(
                    x1[:], c1_t[(n, h)][:],
                    mybir.ActivationFunctionType.Identity,
                    bias=b1v[:, h:h + 1], scale=s1v[:, h:h + 1])
                nc.vector.tensor_add(
                    x1[:], x1[:], m_t[(n, h)].rearrange("p a b -> p (a b)"))
                x1_t[(n, h)] = x1
                sx1 = sx1p.tile([128, NPIX], BF16, name="sx1")
                nc.scalar.sign(sx1[:], x1[:])
                sx1_t[(n, h)] = sx1

            def stage_c2(co, n):
                ps2 = pp.tile([128, 1024], F32, name="ps2", tag="ps")
                for ci in range(2):
                    woff = (ci * 2 + co) * 128
                    for cc0, ccn in ((0, 512), (512, NPIX - 512)):
                        nc.tensor.matmul(
                            ps2[:, cc0:cc0 + ccn],
                            w2t[:, woff:woff + 128],
                            sx1_t[(n, ci)][:, cc0:cc0 + ccn],
                            start=(ci == 0), stop=(ci == 1))
                for q in range(2):
                    nc.vector.bn_stats(
                        st2[:, co, 2 * n + q], ps2[:, 392 * q:392 * q + 392])
                if co == 0:
                    # co=0 psum slots are needed by conv2(co=1): evict to fp16
                    c2 = c2p.tile([128, NPIX], F16, name="c2")
                    nc.scalar.copy(c2[:], ps2[:, 0:NPIX])
                    c2_t[(n, co)] = c2
                else:
                    # co=1 psums are the last PSUM users: keep resident and
                    # let phase E read them directly
                    c2_t[(n, co)] = ps2[:, 0:NPIX]

            def ar2():
              with tc.high_priority():
                for co in range(2):
                    nc.vector.bn_aggr(agg2[:, co], st2[:, co])
                    nc.vector.tensor_scalar_mul(
                        pk2[:, co, 0:1], agg2[:, co, 0:1], float(NLOC))
                    tq2 = vp.tile([128, 1], F32, name="tq2", tag=f"tq2{co}")
                    nc.vector.tensor_mul(tq2[:], agg2[:, co, 0:1], agg2[:, co, 0:1])
                    nc.vector.tensor_add(tq2[:], tq2[:], agg2[:, co, 1:2])
                    nc.vector.tensor_scalar_mul(pk2[:, co, 1:2], tq2[:], float(NLOC))
                ain = dramp.tile([128, 4], F32, name="ar2in")
                aout = dramp.tile([128, 4], F32, name="ar2out")
                nc.sync.dma_start(ain[:], pk2.rearrange("p a b -> p (a b)"))
                nc.gpsimd.collective_compute(
                    "AllReduce", mybir.AluOpType.add, replica_groups=RG,
                    ins=[ain.opt()], outs=[aout.opt()])
                nc.sync.dma_start(g2pk.rearrange("p a b -> p (a b)"), aout[:])
                _affine_from_sq(
                    nc, vp, g2pk[:, :, 0], g2pk[:, :, 1],
                    vecs[:, VG2:VG2 + 3:2], vecs[:, VB2:VB2 + 3:2],
                    s2v[:], b2e[:], tag="a2", w=2)

            def stage_e(co, n):
                ot = outp.tile([128, NPIX], F32, name="ot")
                nc.scalar.activation(
                    ot[:], c2_t[(n, co)][:],
                    mybir.ActivationFunctionType.Identity,
                    bias=b2e[:, co:co + 1], scale=s2v[:, co:co + 1])
                nc.vector.tensor_add(ot[:], ot[:], x1_t[(n, co)][:])
                nc.sync.dma_start(
                    out_ap[n, 128 * co:128 * co + 128],
                    ot.rearrange("p (a b) -> p a b", a=OH))

            for n in range(BL):
                stage_c1(0, n)
                stage_c1(1, n)
            for n in range(BL):
                stage_c2(0, n)
            for n in range(BL):
                stage_c2(1, n)
            ar2()
            for n in range(BL):
                stage_e(0, n)
            for n in range(BL):
                stage_e(1, n)

    nc.compile()
    return nc


_NC = None


def _get_nc():
    global _NC
    if _NC is None:
        _NC = _build()
    return _NC


def _prep_inputs(x, w1, g1, b1, w2, g2, b2):
    """Host-side weight binarization + layout packing (weights are tiny)."""
    x = np.ascontiguousarray(x, dtype=np.float32)

    sw1 = np.sign(w1.astype(np.float32))            # [256, 64, 3, 3]
    t1 = np.zeros((128, 3, 3, 2, 128), np.float32)  # [cin_l, kh, kw, h, cout_l]
    for h in range(2):
        for bb in range(2):
            blk = sw1[128 * h + 64 * bb:128 * h + 64 * bb + 64]  # [64co,64ci,3,3]
            t1[64 * bb:64 * bb + 64, :, :, h, 64 * bb:64 * bb + 64] = \
                blk.transpose(1, 2, 3, 0)
    w1bv = t1.reshape(128, 2304).astype(ml_dtypes.bfloat16)

    sw2 = np.sign(w2.astype(np.float32)[:, :, 0, 0])  # [256 cout, 256 cin]
    t2 = np.zeros((128, 2, 2, 128), np.float32)       # [cin_l, ci, co, cout_l]
    for ci in range(2):
        for co in range(2):
            t2[:, ci, co, :] = sw2[128 * co:128 * co + 128,
                                   128 * ci:128 * ci + 128].T
    w2bv = t2.reshape(128, 512).astype(ml_dtypes.bfloat16)

    k2 = sw2.sum(axis=1).astype(np.float32)           # [256]
    vecs = np.zeros((128, 12), np.float32)
    vecs[:, VG1] = g1[:128]
    vecs[:, VB1] = b1[:128]
    vecs[:, VG1 + 2] = g1[128:]
    vecs[:, VB1 + 2] = b1[128:]
    vecs[:, VG2] = g2[:128]
    vecs[:, VB2] = b2[:128]
    vecs[:, VG2 + 2] = g2[128:]
    vecs[:, VB2 + 2] = b2[128:]
    vecs[:, VK2] = k2[:128]
    vecs[:, VK2 + 1] = k2[128:]
    vecs[:, VK2B] = NGLB * k2[:128]
    vecs[:, VK2B + 1] = NGLB * k2[128:]

    in_maps = []
    for i in range(N_CORES):
        in_maps.append({
            "xs": np.ascontiguousarray(x[BL * i:BL * (i + 1)]),
            "w1b": w1bv,
            "w2b": w2bv,
            "vecs": vecs,
        })
    return in_maps


def run(x, w1, g1, b1, w2, g2, b2, trace=False):
    nc = _get_nc()
    in_maps = _prep_inputs(x, w1, g1, b1, w2, g2, b2)
    res = bass_utils.run_bass_kernel_spmd(
        nc, in_maps, core_ids=list(range(N_CORES)), trace=trace)
    out = np.concatenate([res.results[i]["out"] for i in range(N_CORES)], axis=0)
    return out, res


def kernel(**inputs):
    out, _ = run(
        inputs["x"], inputs["w1"], inputs["g1"], inputs["b1"],
        inputs["w2"], inputs["g2"], inputs["b2"])
    return out



# revision 4
# speedup vs baseline: 1.1179x; 1.1179x over previous
"""Trainium2 Bass kernel for nn_BinaryDilGroupConv.

Reference computation (B=32, C=256, H=W=56, GROUPS=4):
    c1  = conv2d(sign(x), sign(w1), stride=2, pad=1, groups=4)   # -> (B,256,28,28)
    x1  = batchnorm_train(c1, g1, b1) + maxpool3x3s2p1(x)
    c2  = conv2d(sign(x1), sign(w2), 1x1)
    out = batchnorm_train(c2, g2, b2) + x1

Strategy: data-parallel over batch across 8 NeuronCores (4 images/core).
BatchNorm batch statistics are all-reduced across cores (one tiny
[128,4] f32 AllReduce per stage); a dummy warmup AllReduce issued at
kernel start absorbs the collective path's cold-start / multi-core
launch-skew latency concurrently with the input DMA + conv1 phase.
Weights are sign-binarized, transposed and block-diag packed on the host
(they are tiny).  conv1 consumes a true sign(x) in bf16 (exact +-1/0);
conv2 consumes a true sign(x1) produced by the Scalar engine (exact
because all conv accumulations are small integers held in fp32 PSUM).
Phase C is balanced across engines: ACT does the BN1 affine and the
sign, DVE adds the maxpool shortcut and computes conv2 stats, PE runs
conv2; the second output-half's conv2 results stay PSUM-resident and
are consumed directly by the output affine.
"""

import sys

for _p in ("/opt/trn_rl_repo", "/root/.axon_site/_ro/trn_rl_repo"):
    if _p not in sys.path:
        sys.path.append(_p)

import numpy as np
import ml_dtypes

import concourse.bass as bass
import concourse.bacc as bacc
import concourse.mybir as mybir
import concourse.tile as tile
from concourse import bass_utils

N_CORES = 8
B, C, H, W = 32, 256, 56, 56
BL = B // N_CORES          # images per core
OH = OW = 28
NPIX = OH * OW             # 784
NLOC = BL * NPIX           # samples/channel for local stats (3136)
NGLB = B * NPIX            # samples/channel for global stats (25088)
EPS = 1e-5

F32 = mybir.dt.float32
F16 = mybir.dt.float16
BF16 = mybir.dt.bfloat16

# conv1 kernel-position order: center first (start=True covers the full
# output rect), (2,2) last (stop=True covers the full rect too).
KPOS_ORDER = [(1, 1), (0, 0), (0, 1), (0, 2), (1, 0), (1, 2), (2, 0), (2, 1), (2, 2)]

# vecs columns
VG1, VB1 = 0, 1            # +2*h
VG2, VB2 = 4, 5            # +2*co
VK2, VK2B = 8, 10          # +co

RG = [list(range(N_CORES))]


def _emit_conv1(nc, ps, sx, w1t, h):
    for kh, kw in KPOS_ORDER:
        i0 = 1 if kh == 0 else 0
        j0 = 1 if kw == 0 else 0
        ncol = 28 - j0
        woff = ((kh * 3 + kw) * 2 + h) * 128
        for b in range(2):
            r0 = max(i0, 14 * b)
            nr = 14 * b + 14 - r0
            a0 = 2 * r0 + kh - 1
            c0 = 2 * j0 + kw - 1
            bank = ps[:, 512 * b:512 * b + 392].rearrange("p (r c) -> p r c", c=28)
            out_v = bank[:, r0 - 14 * b:r0 - 14 * b + nr, j0:28]
            rhs = sx[:, a0:a0 + 2 * (nr - 1) + 1:2, c0:c0 + 2 * (ncol - 1) + 1:2]
            nc.tensor.matmul(
                out_v,
                w1t[:, woff:woff + 128],
                rhs,
                start=(kh == 1 and kw == 1),
                stop=(kh == 2 and kw == 2),
            )


def _affine_from_sq(nc, vp, s_ap, q_ap, g_ap, b_ap, s_out, b_out, tag,
                    w=1):
    """s_out/b_out [128,w]: BN affine from global sum/sumsq."""
    mg = vp.tile([128, w], F32, name="mg", tag=f"mg{tag}")
    nc.vector.tensor_scalar_mul(mg[:], s_ap, 1.0 / NGLB)
    e2 = vp.tile([128, w], F32, name="e2", tag=f"e2{tag}")
    nc.vector.tensor_scalar_mul(e2[:], q_ap, 1.0 / NGLB)
    m2 = vp.tile([128, w], F32, name="m2", tag=f"m2{tag}")
    nc.vector.tensor_mul(m2[:], mg[:], mg[:])
    vr = vp.tile([128, w], F32, name="vr", tag=f"vr{tag}")
    nc.vector.tensor_sub(vr[:], e2[:], m2[:])
    nc.vector.tensor_scalar_add(vr[:], vr[:], EPS)
    sd = vp.tile([128, w], F32, name="sd", tag=f"sd{tag}")
    nc.scalar.sqrt(sd[:], vr[:])
    inv = vp.tile([128, w], F32, name="inv", tag=f"inv{tag}")
    nc.vector.reciprocal(inv[:], sd[:])
    nc.vector.tensor_mul(s_out, inv[:], g_ap)
    t2 = vp.tile([128, w], F32, name="t2", tag=f"t2{tag}")
    nc.vector.tensor_mul(t2[:], mg[:], s_out)
    nc.vector.tensor_sub(b_out, b_ap, t2[:])


def _build():
    nc = bacc.Bacc(
        "TRN2",
        target_bir_lowering=False,
        debug=False,
        enable_asserts=False,
        num_devices=N_CORES,
    )
    xs = nc.dram_tensor("xs", [BL, C, H, W], F32, kind="ExternalInput")
    w1b = nc.dram_tensor("w1b", [128, 2304], BF16, kind="ExternalInput")
    w2b = nc.dram_tensor("w2b", [128, 512], BF16, kind="ExternalInput")
    vecs_d = nc.dram_tensor("vecs", [128, 12], F32, kind="ExternalInput")
    out_d = nc.dram_tensor("out", [BL, C, OH, OW], F32, kind="ExternalOutput")

    xs_ap = xs.ap()
    out_ap = out_d.ap()

    with tile.TileContext(nc) as tc:
        with tc.tile_pool(name="wp", bufs=1) as wp, \
             tc.tile_pool(name="xp", bufs=3) as xp, \
             tc.tile_pool(name="sxp", bufs=3) as sxp, \
             tc.tile_pool(name="tp", bufs=2) as tp, \
             tc.tile_pool(name="mp", bufs=8) as mp, \
             tc.tile_pool(name="c1p", bufs=8) as c1p, \
             tc.tile_pool(name="x1p", bufs=8) as x1p, \
             tc.tile_pool(name="sx1p", bufs=8) as sx1p, \
             tc.tile_pool(name="c2p", bufs=8) as c2p, \
             tc.tile_pool(name="outp", bufs=6) as outp, \
             tc.tile_pool(name="vp", bufs=2) as vp, \
             tc.tile_pool(name="pp", bufs=4, space="PSUM") as pp, \
             tc.tile_pool(name="dramp", bufs=1, space="DRAM") as dramp:

            # ---- constants ----
            w1t = wp.tile([128, 2304], BF16)
            nc.scalar.dma_start(w1t[:], w1b.ap())
            w2t = wp.tile([128, 512], BF16)
            nc.scalar.dma_start(w2t[:], w2b.ap())
            vecs = wp.tile([128, 12], F32)
            nc.scalar.dma_start(vecs[:], vecs_d.ap())

            # stats: [p, half/co, chunk(2n+b), 6]
            st1 = wp.tile([128, 2, 8, 6], F32)
            st2 = wp.tile([128, 2, 8, 6], F32)
            s1v = wp.tile([128, 2], F32)
            b1v = wp.tile([128, 2], F32)
            s2v = wp.tile([128, 2], F32)
            b2e = wp.tile([128, 2], F32)
            agg1 = wp.tile([128, 2, 2], F32)
            agg2 = wp.tile([128, 2, 2], F32)
            pk1 = wp.tile([128, 2, 2], F32)
            pk2 = wp.tile([128, 2, 2], F32)
            g1pk = wp.tile([128, 2, 2], F32)
            g2pk = wp.tile([128, 2, 2], F32)

            m_t = {}
            c1_t = {}
            x1_t = {}
            sx1_t = {}
            c2_t = {}

            # ======== phase A (per half h): load, sign, maxpool, conv1,
            # stats -> per-half AllReduce overlapped with the other half ====
            def stage_a(h, n):
                xt = xp.tile([128, H, W], F32, name="xt")
                nc.sync.dma_start(xt[:], xs_ap[n, 128 * h:128 * h + 128])
                sx = sxp.tile([128, H, W], BF16, name="sx")
                nc.scalar.sign(sx[:], xt[:])

                # maxpool 3x3 s2 p1 (separable, boundary-safe)
                t = tp.tile([128, OH, W], F32, name="t")
                nc.vector.tensor_max(t[:], xt[:, 0:56:2, :], xt[:, 1:56:2, :])
                nc.vector.tensor_max(t[:, 1:28, :], t[:, 1:28, :], xt[:, 1:54:2, :])
                m = mp.tile([128, OH, OW], F32, name="m")
                nc.vector.tensor_max(m[:], t[:, :, 0:56:2], t[:, :, 1:56:2])
                nc.vector.tensor_max(m[:, :, 1:28], m[:, :, 1:28], t[:, :, 1:54:2])
                m_t[(n, h)] = m

                ps = pp.tile([128, 1024], F32, name="ps", tag="ps")
                _emit_conv1(nc, ps, sx, w1t, h)
                for b in range(2):
                    nc.vector.bn_stats(
                        st1[:, h, 2 * n + b], ps[:, 512 * b:512 * b + 392])
                c1 = c1p.tile([128, NPIX], F16, name="c1")
                nc.scalar.copy(
                    c1.rearrange("p (b x) -> p b x", b=2),
                    ps.rearrange("p (b x) -> p b x", b=2)[:, :, 0:392])
                c1_t[(n, h)] = c1

            def ar1():
              with tc.high_priority():
                for h in range(2):
                    nc.vector.bn_aggr(agg1[:, h], st1[:, h])
                    nc.vector.tensor_scalar_mul(
                        pk1[:, h, 0:1], agg1[:, h, 0:1], float(NLOC))
                    tq = vp.tile([128, 1], F32, name="tq", tag=f"tq1{h}")
                    nc.vector.tensor_mul(tq[:], agg1[:, h, 0:1], agg1[:, h, 0:1])
                    nc.vector.tensor_add(tq[:], tq[:], agg1[:, h, 1:2])
                    nc.vector.tensor_scalar_mul(pk1[:, h, 1:2], tq[:], float(NLOC))
                ain = dramp.tile([128, 4], F32, name="ar1in", tag="ar1in")
                aout = dramp.tile([128, 4], F32, name="ar1out", tag="ar1out")
                nc.sync.dma_start(ain[:], pk1.rearrange("p a b -> p (a b)"))
                nc.gpsimd.collective_compute(
                    "AllReduce", mybir.AluOpType.add, replica_groups=RG,
                    ins=[ain.opt()], outs=[aout.opt()])
                nc.sync.dma_start(g1pk.rearrange("p a b -> p (a b)"), aout[:])
                _affine_from_sq(
                    nc, vp, g1pk[:, :, 0], g1pk[:, :, 1],
                    vecs[:, VG1:VG1 + 3:2], vecs[:, VB1:VB1 + 3:2],
                    s1v[:], b1v[:], tag="a1", w=2)

            for h in range(2):
                for n in range(BL):
                    stage_a(h, n)
            ar1()

            # ======== phase C: x1' = s1*c1 + m, sx1 = {0,2}(x1'>=-b1'),
            # conv2 per output half, per-half stats AllReduce ==============
            def stage_c1(h, n):
                # x1 = s1*c1 + b1 + m: ACT does the affine, DVE adds the
                # shortcut, ACT produces the exact bf16 sign for conv2.
                x1 = x1p.tile([128, NPIX], F32, name="x1")
                nc.scalar.activation(
                    x1[:], c1_t[(n, h)][:],
                    mybir.ActivationFunctionType.Identity,
                    bias=b1v[:, h:h + 1], scale=s1v[:, h:h + 1])
                nc.vector.tensor_add(
                    x1[:], x1[:], m_t[(n, h)].rearrange("p a b -> p (a b)"))
                x1_t[(n, h)] = x1
                sx1 = sx1p.tile([128, NPIX], BF16, name="sx1")
                nc.scalar.sign(sx1[:], x1[:])
                sx1_t[(n, h)] = sx1

            def stage_c2(co, n):
                ps2 = pp.tile([128, 1024], F32, name="ps2", tag="ps")
                for ci in range(2):
                    woff = (ci * 2 + co) * 128
                    for cc0, ccn in ((0, 512), (512, NPIX - 512)):
                        nc.tensor.matmul(
                            ps2[:, cc0:cc0 + ccn],
                            w2t[:, woff:woff + 128],
                            sx1_t[(n, ci)][:, cc0:cc0 + ccn],
                            start=(ci == 0), stop=(ci == 1))
                for q in range(2):
                    nc.vector.bn_stats(
                        st2[:, co, 2 * n + q], ps2[:, 392 * q:392 * q + 392])
                if co == 0:
                    # co=0 psum slots are needed by conv2(co=1): evict to fp16
                    c2 = c2p.tile([128, NPIX], F16, name="c2")
                    nc.scalar.copy(c2[:], ps2[:, 0:NPIX])
                    c2_t[(n, co)] = c2
                else:
                    # co=1 psums are the last PSUM users: keep resident and
                    # let phase E read them directly
                    c2_t[(n, co)] = ps2[:, 0:NPIX]

            def ar2():
              with tc.high_priority():
                for co in range(2):
                    nc.vector.bn_aggr(agg2[:, co], st2[:, co])
                    nc.vector.tensor_scalar_mul(
                        pk2[:, co, 0:1], agg2[:, co, 0:1], float(NLOC))
                    tq2 = vp.tile([128, 1], F32, name="tq2", tag=f"tq2{co}")
                    nc.vector.tensor_mul(tq2[:], agg2[:, co, 0:1], agg2[:, co, 0:1])
                    nc.vector.tensor_add(tq2[:], tq2[:], agg2[:, co, 1:2])
                    nc.vector.tensor_scalar_mul(pk2[:, co, 1:2], tq2[:], float(NLOC))
                ain = dramp.tile([128, 4], F32, name="ar2in", tag="ar2in")
                aout = dramp.tile([128, 4], F32, name="ar2out", tag="ar2out")
                nc.sync.dma_start(ain[:], pk2.rearrange("p a b -> p (a b)"))
                nc.gpsimd.collective_compute(
                    "AllReduce", mybir.AluOpType.add, replica_groups=RG,
                    ins=[ain.opt()], outs=[aout.opt()])
                nc.sync.dma_start(g2pk.rearrange("p a b -> p (a b)"), aout[:])
                _affine_from_sq(
                    nc, vp, g2pk[:, :, 0], g2pk[:, :, 1],
                    vecs[:, VG2:VG2 + 3:2], vecs[:, VB2:VB2 + 3:2],
                    s2v[:], b2e[:], tag="a2", w=2)

            def stage_e(co, n):
                ot = outp.tile([128, NPIX], F32, name="ot")
                nc.scalar.activation(
                    ot[:], c2_t[(n, co)][:],
                    mybir.ActivationFunctionType.Identity,
                    bias=b2e[:, co:co + 1], scale=s2v[:, co:co + 1])
                nc.vector.tensor_add(ot[:], ot[:], x1_t[(n, co)][:])
                nc.sync.dma_start(
                    out_ap[n, 128 * co:128 * co + 128],
                    ot.rearrange("p (a b) -> p a b", a=OH))

            for n in range(BL):
                stage_c1(0, n)
                stage_c1(1, n)
            for n in range(BL):
                stage_c2(0, n)
            for n in range(BL):
                stage_c2(1, n)
            ar2()
            for n in range(BL):
                stage_e(0, n)
            for n in range(BL):
                stage_e(1, n)

    nc.compile()
    return nc


_NC = None


def _get_nc():
    global _NC
    if _NC is None:
        _NC = _build()
    return _NC


def _prep_inputs(x, w1, g1, b1, w2, g2, b2):
    """Host-side weight binarization + layout packing (weights are tiny)."""
    x = np.ascontiguousarray(x, dtype=np.float32)

    sw1 = np.sign(w1.astype(np.float32))            # [256, 64, 3, 3]
    t1 = np.zeros((128, 3, 3, 2, 128), np.float32)  # [cin_l, kh, kw, h, cout_l]
    for h in range(2):
        for bb in range(2):
            blk = sw1[128 * h + 64 * bb:128 * h + 64 * bb + 64]  # [64co,64ci,3,3]
            t1[64 * bb:64 * bb + 64, :, :, h, 64 * bb:64 * bb + 64] = \
                blk.transpose(1, 2, 3, 0)
    w1bv = t1.reshape(128, 2304).astype(ml_dtypes.bfloat16)

    sw2 = np.sign(w2.astype(np.float32)[:, :, 0, 0])  # [256 cout, 256 cin]
    t2 = np.zeros((128, 2, 2, 128), np.float32)       # [cin_l, ci, co, cout_l]
    for ci in range(2):
        for co in range(2):
            t2[:, ci, co, :] = sw2[128 * co:128 * co + 128,
                                   128 * ci:128 * ci + 128].T
    w2bv = t2.reshape(128, 512).astype(ml_dtypes.bfloat16)

    k2 = sw2.sum(axis=1).astype(np.float32)           # [256]
    vecs = np.zeros((128, 12), np.float32)
    vecs[:, VG1] = g1[:128]
    vecs[:, VB1] = b1[:128]
    vecs[:, VG1 + 2] = g1[128:]
    vecs[:, VB1 + 2] = b1[128:]
    vecs[:, VG2] = g2[:128]
    vecs[:, VB2] = b2[:128]
    vecs[:, VG2 + 2] = g2[128:]
    vecs[:, VB2 + 2] = b2[128:]
    vecs[:, VK2] = k2[:128]
    vecs[:, VK2 + 1] = k2[128:]
    vecs[:, VK2B] = NGLB * k2[:128]
    vecs[:, VK2B + 1] = NGLB * k2[128:]

    in_maps = []
    for i in range(N_CORES):
        in_maps.append({
            "xs": np.ascontiguousarray(x[BL * i:BL * (i + 1)]),
            "w1b": w1bv,
            "w2b": w2bv,
            "vecs": vecs,
        })
    return in_maps


def run(x, w1, g1, b1, w2, g2, b2, trace=False):
    nc = _get_nc()
    in_maps = _prep_inputs(x, w1, g1, b1, w2, g2, b2)
    res = bass_utils.run_bass_kernel_spmd(
        nc, in_maps, core_ids=list(range(N_CORES)), trace=trace)
    out = np.concatenate([res.results[i]["out"] for i in range(N_CORES)], axis=0)
    return out, res


def kernel(**inputs):
    out, _ = run(
        inputs["x"], inputs["w1"], inputs["g1"], inputs["b1"],
        inputs["w2"], inputs["g2"], inputs["b2"])
    return out

